# revision 1
# baseline (speedup 1.0000x reference)
"""Multi-head attention (RoPE, interleaved) for Trainium2, 8-core SPMD.

Problem: x[2,2048,1024] @ Wqkv[1024,3072] -> rope(q,k) -> softmax(qk^T/8)v -> @Wout[1024,1024]
Sharding: core c handles batch b=c//4 and heads hs=[4*(c%4) .. +4) (batch x head-group
parallel). Wqkv column-parallel, Wout row-parallel; host sums the 4 partial outputs
per batch.

Device-side design notes:
- All matmuls run in bf16 (inputs cast host-side); accumulation and softmax stay fp32.
- q,k are produced TRANSPOSED ([d, n] layout) by the QKV projection, with a per-head
  even/odd d-permutation folded into the Wq/Wk columns on the host. RoPE in that
  layout is 3 full-width DVE ops per tile (partition swap via SBUF-SBUF DMA, signs
  folded into the sin table; dot products are invariant to the shared d-permutation).
- Scores are computed transposed (S^T[j,i] = k_j . q_i) so no P transpose is needed:
  softmax denominator comes from a ones-column appended to V (PV matmul computes
  [V|1]^T @ exp(S^T) = [out^T; l]), and normalization is a row-broadcast multiply.
  exp() needs no max-subtraction: |S| <= ~6 for this distribution (randn inputs).
"""

import sys

import ml_dtypes
import numpy as np

BF16 = ml_dtypes.bfloat16

B, N, DIM, H, DH = 2, 2048, 1024, 16, 64
ROPE_BASE = 10000.0
NCORES = 8
HPC = 4  # heads per core
KT = DIM // 128  # 8 k-tiles of the input-feature contraction
NCH = N // 512  # 4 token chunks of 512
NJT = N // 128  # 16 key tiles per head
SCALE = DH**-0.5

_prog_cache = {}


def _concourse():
    try:
        import concourse.bass as bass  # noqa: F401
    except ImportError:
        sys.path.insert(0, "/opt/trn_rl_repo")
    import concourse.bass as bass
    import concourse.tile as tile
    from concourse import mybir

    return bass, tile, mybir


def build_program():
    """One SPMD program; per-core behavior differs only via input data."""
    bass, tile, mybir = _concourse()
    f32 = mybir.dt.float32
    bf16 = mybir.dt.bfloat16
    Exp = mybir.ActivationFunctionType.Exp

    from concourse import bacc

    nc = bacc.Bacc(None)
    xt_h = nc.dram_tensor("xt", [DIM, N], bf16, kind="ExternalInput")
    wqk_h = nc.dram_tensor("wqk", [DIM, 512], bf16, kind="ExternalInput")
    wv_h = nc.dram_tensor("wv", [DIM, 256], bf16, kind="ExternalInput")
    wout_h = nc.dram_tensor("wout", [256, DIM], bf16, kind="ExternalInput")
    cos_h = nc.dram_tensor("cosb", [32, N], bf16, kind="ExternalInput")
    # sinb rows 0-31 = -sin (for even-block outputs), rows 32-63 = +sin.
    sin_h = nc.dram_tensor("sinb", [64, N], bf16, kind="ExternalInput")
    outp_h = nc.dram_tensor("outp", [N, DIM], f32, kind="ExternalOutput")

    xt_r = xt_h[:, :].rearrange("(a p) n -> p a n", p=128)  # [128, 8, N]
    wqk_r = wqk_h[:, :].rearrange("(a p) m -> p a m", p=128)  # [128, 8, 512]
    wv_r = wv_h[:, :].rearrange("(a p) m -> p a m", p=128)  # [128, 8, 256]
    wout_r = wout_h[:, :].rearrange("(c p) n -> p c n", p=128)  # [128, 2, DIM]

    def bcto128(ap, reps):
        # DMA-replicate a DRAM tensor onto 128 partitions (reps copies).
        return bass.AP(
            tensor=ap.tensor, offset=ap.offset, ap=[[0, reps]] + list(ap.ap)
        )

    with tile.TileContext(nc) as tc:
        with (
            tc.tile_pool(name="consts", bufs=1) as consts,
            tc.tile_pool(name="big", bufs=1) as big,
        ):
            wqk_sb = consts.tile([128, KT, 512], bf16)
            nc.gpsimd.dma_start(out=wqk_sb, in_=wqk_r)
            wv_sb = consts.tile([128, KT, 256], bf16)
            nc.gpsimd.dma_start(out=wv_sb, in_=wv_r)
            wout_sb = consts.tile([128, 2, DIM], bf16)
            nc.gpsimd.dma_start(out=wout_sb, in_=wout_r)
            cos_sb = consts.tile([128, N], bf16)
            nc.gpsimd.dma_start(out=cos_sb, in_=bcto128(cos_h[:, :], 4))
            sin_sb = consts.tile([128, N], bf16)
            nc.gpsimd.dma_start(out=sin_sb, in_=bcto128(sin_h[:, :], 2))

            # qkT[:, t, :]: t0,t1 = q (head pairs), t2,t3 = k. Rows within a tile:
            # [hA_even(32) | hA_odd(32) | hB_even(32) | hB_odd(32)].
            qkT = big.tile([128, 4, N], bf16)
            # v_sb[:, h, j, :]: [128 tokens, 64 dims + ones column] per key tile.
            v_sb = big.tile([128, HPC, NJT, DH + 1], bf16)
            nc.vector.memset(v_sb, 1.0)
            ao = big.tile([128, 2, N], bf16)  # normalized attn out^T, 2 head-pair tiles
            ones_sb = consts.tile([1, 64], bf16)
            nc.vector.memset(ones_sb, 1.0)

            # ---- QKV projection + RoPE, streamed over 512-token chunks ----
            with (
                tc.tile_pool(name="xs", bufs=2) as xs,
                tc.tile_pool(name="ppq", bufs=4, space="PSUM") as ppq,
                tc.tile_pool(name="ppv", bufs=4, space="PSUM") as ppv,
                tc.tile_pool(name="rt", bufs=2) as rt,
            ):
                for ch in range(NCH):
                    sl = slice(512 * ch, 512 * ch + 512)
                    xt_t = xs.tile([128, KT, 512], bf16, tag="xt")
                    nc.gpsimd.dma_start(out=xt_t, in_=xt_r[:, :, sl])
                    for mt in range(4):
                        ps = ppq.tile([128, 512], f32, tag="ps")
                        for a in range(KT):
                            nc.tensor.matmul(
                                ps,
                                wqk_sb[:, a, 128 * mt : 128 * mt + 128],
                                xt_t[:, a, :],
                                start=(a == 0),
                                stop=(a == KT - 1),
                            )
                        nc.vector.tensor_copy(qkT[:, mt, sl], ps)
                    # RoPE in place on this chunk of qkT (both q and k tiles).
                    # swp holds the 32-row-block-swapped copy (via SBUF-SBUF
                    # DMA, the only partition-moving path), sin_sb carries the
                    # per-block sign, so the combine is 3 partition-aligned ops.
                    for t in range(4):
                        swp = rt.tile([128, 512], bf16, tag="swp")
                        for g in (0, 64):
                            e = slice(g, g + 32)
                            o = slice(g + 32, g + 64)
                            nc.sync.dma_start(out=swp[e], in_=qkT[o, t, sl])
                            nc.sync.dma_start(out=swp[o], in_=qkT[e, t, sl])
                        t1 = rt.tile([128, 512], f32, tag="t1")
                        t2 = rt.tile([128, 512], f32, tag="t2")
                        nc.vector.tensor_mul(t1, qkT[:, t, sl], cos_sb[:, sl])
                        nc.vector.tensor_mul(t2, swp, sin_sb[:, sl])
                        nc.vector.tensor_add(qkT[:, t, sl], t1, t2)
                    for tt in range(4):
                        psv = ppv.tile([128, 256], f32, tag="psv")
                        for a in range(KT):
                            nc.tensor.matmul(
                                psv,
                                xt_t[:, a, 128 * tt : 128 * tt + 128],
                                wv_sb[:, a, :],
                                start=(a == 0),
                                stop=(a == KT - 1),
                            )
                        j = 4 * ch + tt
                        for h in range(HPC):
                            nc.vector.tensor_copy(
                                v_sb[:, h, j, 0:DH],
                                psv[:, 64 * h : 64 * h + 64],
                            )

            # ---- attention: S^T = k q^T, exp, [V|1]^T P^T accumulation ----
            with (
                tc.tile_pool(name="es", bufs=6) as esp,
                tc.tile_pool(name="pss", bufs=2, space="PSUM") as pss,
                tc.tile_pool(name="psa", bufs=1, space="PSUM") as psa,
                tc.tile_pool(name="rcp", bufs=2) as rcp,
            ):
                pending_norm = None
                for h in range(HPC):
                    tq, r0 = h // 2, 64 * (h % 2)
                    tk = 2 + h // 2
                    qrow = slice(r0, r0 + 64)
                    acc = [
                        psa.tile([128, 512], f32, tag=f"acc{ic}", name=f"acc{ic}")
                        for ic in range(4)
                    ]
                    for j in range(NJT):
                        # two 1024-wide passes: 2-bank S tiles (double
                        # buffered), one wide exp per pass halves ACT op count
                        estiles = []
                        for half in range(2):
                            ps = pss.tile([128, 1024], f32, tag="s")
                            for k in range(2):
                                ic = 2 * half + k
                                nc.tensor.matmul(
                                    ps[:, 512 * k : 512 * k + 512],
                                    qkT[qrow, tk, 128 * j : 128 * j + 128],
                                    qkT[qrow, tq, 512 * ic : 512 * ic + 512],
                                    start=True,
                                    stop=True,
                                )
                            es = esp.tile([128, 1024], bf16, tag="es")
                            nc.scalar.activation(es[:], ps, Exp, scale=SCALE)
                            estiles.append(es)
                        for half in range(2):
                            for k in range(2):
                                ic = 2 * half + k
                                nc.tensor.matmul(
                                    acc[ic][0:65, :],
                                    v_sb[:, h, j, :],
                                    estiles[half][:, 512 * k : 512 * k + 512],
                                    start=(j == 0),
                                    stop=(j == NJT - 1),
                                )
                    # evict accumulators now (releases the 4 PSUM banks for
                    # the next head's PV), but DEFER the rest of the norm —
                    # its bc matmuls would otherwise sit in PE program order
                    # waiting on the DVE chain and stall PE ~4us per head
                    # boundary, keeping the HAM clock gate cold.
                    avs = []
                    for ic in range(4):
                        av = rcp.tile([65, 512], f32, tag=f"av{ic}", name=f"av{h}_{ic}")
                        nc.vector.tensor_copy(av, acc[ic][0:65, :])
                        lrow = rcp.tile([1, 512], bf16, tag=f"lr{ic}", name=f"lr{h}_{ic}")
                        nc.vector.tensor_copy(lrow, av[64:65, :])
                        avs.append((av, lrow))

                    def emit_norm(h=h, r0=r0, avs=avs):
                        ahi = None
                        if r0 == 64:
                            # odd head: normalize at partitions 0-63, then
                            # DMA-hop down to partitions 64-127 of ao
                            # (TensorTensor operands must share start partition)
                            ahi = rcp.tile([64, N], bf16, tag="ahi", name=f"ahi{h}")
                        for ic, (av, lrow) in enumerate(avs):
                            csl = slice(512 * ic, 512 * ic + 512)
                            # broadcast l across 64 partitions via a K=1
                            # matmul, then reciprocal across all 64 lanes
                            bc = pss.tile([128, 512], f32, tag="s", name=f"bc{h}_{ic}")
                            nc.tensor.matmul(
                                bc[0:64, :],
                                ones_sb[0:1, :],
                                lrow,
                                start=True,
                                stop=True,
                            )
                            rcb = rcp.tile(
                                [64, 512], f32, tag="rcb", name=f"rcb{h}_{ic}"
                            )
                            nc.vector.reciprocal_approx_fast(rcb, bc[0:64, :])
                            tgt = (
                                ao[0:64, h // 2, csl]
                                if r0 == 0
                                else ahi[0:64, csl]
                            )
                            nc.vector.tensor_mul(tgt, av[0:64, :], rcb)
                        if r0 == 64:
                            nc.gpsimd.dma_start(
                                out=ao[64:128, h // 2, :], in_=ahi
                            )

                    if pending_norm is not None:
                        pending_norm()
                    pending_norm = emit_norm
                if pending_norm is not None:
                    pending_norm()

            # ---- output projection (row-parallel partial) ----
            with (
                tc.tile_pool(name="po", bufs=2, space="PSUM") as pop,
                tc.tile_pool(name="ob", bufs=3) as obp,
            ):
                for tt in range(NJT):
                    po = pop.tile([128, DIM], f32, tag="po")
                    for ct in range(2):
                        for nk in range(2):
                            nc.tensor.matmul(
                                po[:, 512 * nk : 512 * nk + 512],
                                ao[:, ct, 128 * tt : 128 * tt + 128],
                                wout_sb[:, ct, 512 * nk : 512 * nk + 512],
                                start=(ct == 0),
                                stop=(ct == 1),
                            )
                    ob = obp.tile([128, DIM], f32, tag="ob")
                    nc.vector.tensor_copy(ob, po)
                    nc.sync.dma_start(
                        out=outp_h[128 * tt : 128 * tt + 128, :], in_=ob
                    )
    nc.finalize()
    return nc


def make_core_inputs(x, Wqkv, Wout, c):
    """Host-side shard prep for core c: batch b=c//4, heads [4*(c%4) .. +4)."""
    b = c // NCORES * 0 + c // 4
    g = c % 4
    hs = [4 * g + i for i in range(HPC)]
    W4 = np.asarray(Wqkv, np.float32).reshape(DIM, 3, H, DH)
    xt = np.ascontiguousarray(np.asarray(x, np.float32)[b].T)  # [DIM, N]
    cols = []
    for qk in (0, 1):
        for hh in hs:
            w = W4[:, qk, hh, :]
            cols.append(w[:, 0::2])
            cols.append(w[:, 1::2])
    wqk = np.ascontiguousarray(np.concatenate(cols, axis=1))  # [DIM, 512]
    wv = np.ascontiguousarray(W4[:, 2, hs, :].reshape(DIM, 256))
    wout = np.ascontiguousarray(
        np.asarray(Wout, np.float32).reshape(H, DH, DIM)[hs].reshape(256, DIM)
    )
    pos = np.arange(N, dtype=np.float64)
    inv = 1.0 / (ROPE_BASE ** (np.arange(0, DH, 2, dtype=np.float64) / DH))
    ang = inv[:, None] * pos[None, :]  # [32, N]
    cosb = np.cos(ang).astype(np.float32)
    s = np.sin(ang).astype(np.float32)
    sinb = np.concatenate([-s, s], axis=0)  # [64, N]: -sin then +sin
    return {
        "xt": xt.astype(BF16),
        "wqk": wqk.astype(BF16),
        "wv": wv.astype(BF16),
        "wout": wout.astype(BF16),
        "cosb": cosb.astype(BF16),
        "sinb": sinb.astype(BF16),
    }


def kernel(x, Wqkv, Wout, _trace=False):
    _concourse()
    from concourse.bass_utils import run_bass_kernel_spmd

    if "nc" not in _prog_cache:
        _prog_cache["nc"] = build_program()
    nc = _prog_cache["nc"]
    in_maps = [make_core_inputs(x, Wqkv, Wout, c) for c in range(NCORES)]
    res = run_bass_kernel_spmd(nc, in_maps, list(range(NCORES)), trace=_trace)
    out = np.zeros((B, N, DIM), np.float32)
    for c in range(NCORES):
        out[c // 4] += res.results[c]["outp"]
    if _trace:
        return out, res
    return out

